# revision 22
# baseline (speedup 1.0000x reference)
"""CIF (continuous integrate-and-fire) kernel for Trainium2, 8 NeuronCores.

Strategy
--------
The CIF scan over time only has a *scalar* recurrence: the integrate/fire
decisions and the per-step blend weights depend solely on ``alphas`` [B, T]
(256 KB).  All the heavy work involving ``hidden`` [B, T, H] (131 MB) is,
for fixed fire decisions, a linear map: every output frame j is a weighted
sum of consecutive hidden rows,

    out[b, j, :] = sum_t W[b, t, j] * hidden[b, t, :]

where W[b] is a [T, 64] sparse-banded weight matrix (each time step
contributes to at most two adjacent frames; weights are the reference's
``cur``/``remainds`` values).

So: replicate the reference's fp32 scalar scan on the host (exact same op
order -> bit-identical fire decisions), build W, then run the batched
[64, T] @ [T, H] matmul on the 8 NeuronCores — pure data parallel over the
batch dim, 4 rows per core, K-tiled over T with PSUM accumulation.

Device-side layout: the host pre-permutes hidden (and W) into
partition-major chunk layout so every DMA reads long contiguous runs per
SBUF partition, and hidden streams in chunk-groups (small first group so
the PE starts early, bigger later groups for descriptor efficiency) across
both HWDGE rings so the DMA engines stay at the per-core HBM limit.

Matmul dtype (CIF_MM_MODE): default "fp16" — halves the DMA stream vs
fp32 and runs the PE at 1 cycle/row; measured rel err 4.3e-4 on the
fp32 reference (hardware).  "fp32r" (1.6e-4, full fp32 traffic) and
"fp32" (3.6e-7, bit-faithful) are available when tighter accuracy is
preferred over speed: measured 43.2 / ~60 / 85.4 us respectively.
"""

import os

import numpy as np

# --- problem constants (hardcoded per spec: nn_CIF_Model_5970004541927) ---
B, T, H = 32, 2000, 512
NCORES = 8
R = B // NCORES          # batch rows per core = 4
ML = 64                  # MAX_LABELS
THRESH = np.float32(0.95)
P = 128                  # SBUF partitions
NFULL = T // P           # 15 full K-chunks
TAIL = T - NFULL * P     # 80 leftover time steps
NCHUNK = NFULL + 1       # 16
TP = NCHUNK * P          # 2048 (weights padded so chunks divide evenly)
GRP = int(os.environ.get("CIF_GRP", "8"))  # K-chunks per hidden DMA
HBUFS = int(os.environ.get("CIF_HBUFS", "6"))

# matmul dtype on the PE: "fp16" (default), "fp32r", "fp32", or "bf16"
MM_MODE = os.environ.get("CIF_MM_MODE", "fp16")

_COMPILED = {}


def _build_weights(alphas: np.ndarray) -> np.ndarray:
    """Replicate the reference fp32 scan on alphas only.

    Returns WF [B, P, NCHUNK, ML] float32 — the lhsT tiles laid out so the
    device DMA reads one contiguous 4 KB run per partition:
    WF[b, p, c, m] = weight of hidden step t = c*P + p into output frame m.

    Per time step t (exactly the reference ops, vectorized over the batch):
        dist_completion = 1 - integrate
        integrate += a_t ; fire = integrate > 0.95
        integrate -= fire
        cur = fire ? dist_completion : a_t   -> frame n   (n = fires so far)
        remainds = a_t - cur                 -> frame n+1  (only at a fire)
    """
    Bv, Tv = alphas.shape
    a = np.ascontiguousarray(alphas, dtype=np.float32)
    integrate = np.zeros(Bv, np.float32)
    nfires = np.zeros(Bv, np.int64)
    # two dump columns absorb contributions past frame ML-1
    WT = np.zeros((Bv, TP, ML + 2), np.float32)
    rows = np.arange(Bv)
    one = np.float32(1.0)
    for t in range(Tv):
        a_t = a[:, t]
        dist_completion = one - integrate
        integrate = integrate + a_t
        fire = integrate > THRESH
        integrate = np.where(fire, integrate - one, integrate)
        cur = np.where(fire, dist_completion, a_t)
        remainds = a_t - cur
        j = np.minimum(nfires, ML)
        WT[rows, t, j] = cur
        if fire.any():
            fr = rows[fire]
            j2 = np.minimum(nfires[fire] + 1, ML + 1)
            WT[fr, t, j2] = remainds[fire]
        nfires = nfires + fire
    WT = WT[:, :, :ML]                                  # [B, TP, ML]
    WF = WT.reshape(Bv, NCHUNK, P, ML).transpose(0, 2, 1, 3)  # [B, P, NCHUNK, ML]
    return np.ascontiguousarray(WF)


def _build_nc(mm_mode: str):
    """Emit the Bass/Tile program (identical on all 8 cores; SPMD over batch)."""
    import concourse.bacc as bacc
    import concourse.mybir as mybir
    import concourse.tile as tile

    f32 = mybir.dt.float32
    # fp32r: walrus requires matmul operands to be *produced* as float32r,
    # so declare the DRAM tensors and SBUF tiles as float32r throughout.
    in_dt = {
        "fp32": f32,
        "fp32r": mybir.dt.float32r,
        "bf16": mybir.dt.bfloat16,
        "fp16": mybir.dt.float16,
    }[mm_mode]

    nc = bacc.Bacc("TRN2", target_bir_lowering=False, debug=False)
    # hidp: first 1920 steps, partition-major [P, NFULL, H] per row so each
    # DMA group reads one contiguous run per partition.
    hidp = nc.dram_tensor("hidp", [R, P, NFULL * H], in_dt, kind="ExternalInput")
    hidt = nc.dram_tensor("hidt", [R, TAIL, H], in_dt, kind="ExternalInput")
    wt = nc.dram_tensor("wt", [R, P, NCHUNK * ML], in_dt, kind="ExternalInput")
    out = nc.dram_tensor("out", [R, ML, H], f32, kind="ExternalOutput")

    # non-uniform chunk-groups: a small first group lets each row's matmuls
    # start early; bigger middle groups for descriptor efficiency.  The LAST
    # row reverses the pattern so only a tiny group (and ~2 matmuls) remain
    # after the final DMA lands — the whole fleet of DMA engines otherwise
    # sits idle ~4 us behind the serial end-chain.
    def _mk_groups(sizes):
        gs, pos = [], 0
        for s in sizes:
            gs.append(list(range(pos, min(pos + s, NFULL))))
            pos += s
        return gs

    if GRP == 8:
        row_sizes = [[2, 5, 8], [2, 5, 8], [2, 5, 8], [8, 5, 2]]
    else:
        row_sizes = [
            [len(g) for g in _mk_groups([GRP] * ((NFULL + GRP - 1) // GRP))]
        ] * R
    row_groups = [_mk_groups(s) for s in row_sizes]
    gmax = max(len(g) for gs in row_groups for g in gs)

    with tile.TileContext(nc) as tc:
        with (
            tc.tile_pool(name="hpool", bufs=HBUFS) as hpool,
            # R bufs on the small per-row pools: the HWDGE sequencers execute
            # dma_starts in program order, so a tile-slot wait on one DMA
            # head-of-line-blocks descriptor generation for everything behind
            # it on that ring
            tc.tile_pool(name="tpool", bufs=R) as tpool,
            tc.tile_pool(name="wpool", bufs=R) as wpool,
            tc.tile_pool(name="opool", bufs=R) as opool,
            tc.tile_pool(name="psum", bufs=4, space="PSUM") as psum_pool,
        ):
            # two parallel descriptor-generation paths (HWDGE rings)
            rings = [nc.sync, nc.scalar]
            di = 0

            for r in range(R):
                groups = row_groups[r]
                w_tile = wpool.tile([P, NCHUNK * ML], in_dt, tag="w")
                rings[di % 2].dma_start(w_tile[:], wt[r])
                di += 1

                h_tiles = []
                for g in groups:
                    n = len(g)
                    ht = hpool.tile([P, gmax * H], in_dt, tag="h")
                    rings[di % 2].dma_start(
                        ht[:, : n * H], hidp[r][:, g[0] * H : (g[-1] + 1) * H]
                    )
                    di += 1
                    h_tiles.append(ht)
                h_tail = tpool.tile([P, H], in_dt, tag="ht")
                rings[di % 2].dma_start(h_tail[0:TAIL, :], hidt[r])
                di += 1

                ps = psum_pool.tile([ML, H], f32)
                for gi, g in enumerate(groups):
                    ht = h_tiles[gi]
                    for ci, c in enumerate(g):
                        nc.tensor.matmul(
                            ps[:],
                            w_tile[:, c * ML : (c + 1) * ML],
                            ht[:, ci * H : (ci + 1) * H],
                            start=(c == 0),
                            stop=False,
                        )
                nc.tensor.matmul(
                    ps[:],
                    w_tile[0:TAIL, NFULL * ML : NCHUNK * ML],
                    h_tail[0:TAIL, :],
                    start=False,
                    stop=True,
                )
                # PSUM can't source a DMA; bounce through SBUF in two halves
                # so the first out-DMA overlaps the second copy
                o_tile = opool.tile([ML, H], f32, tag="o")
                half = H // 2
                nc.vector.tensor_copy(o_tile[:, :half], ps[:, :half])
                rings[di % 2].dma_start(out[r][:, :half], o_tile[:, :half])
                di += 1
                nc.vector.tensor_copy(o_tile[:, half:], ps[:, half:])
                rings[di % 2].dma_start(out[r][:, half:], o_tile[:, half:])
                di += 1
    nc.compile()
    return nc


def _get_nc(mm_mode: str):
    if mm_mode not in _COMPILED:
        _COMPILED[mm_mode] = _build_nc(mm_mode)
    return _COMPILED[mm_mode]


def kernel(hidden: np.ndarray, alphas: np.ndarray, _trace: bool = False):
    from concourse.bass_utils import run_bass_kernel_spmd

    hidden = np.asarray(hidden, dtype=np.float32)
    alphas = np.asarray(alphas, dtype=np.float32)
    assert hidden.shape == (B, T, H) and alphas.shape == (B, T)

    WF = _build_weights(alphas)  # [B, P, NCHUNK, ML] fp32

    # partition-major repack of the first NFULL*P steps:
    # hidp[b, p, c, h] = hidden[b, c*P + p, h]
    hidp = np.ascontiguousarray(
        hidden[:, : NFULL * P].reshape(B, NFULL, P, H).transpose(0, 2, 1, 3)
    )
    hidt = np.ascontiguousarray(hidden[:, NFULL * P :])

    if MM_MODE == "bf16":
        import ml_dtypes

        hidp = hidp.astype(ml_dtypes.bfloat16)
        hidt = hidt.astype(ml_dtypes.bfloat16)
        WF = WF.astype(ml_dtypes.bfloat16)
    elif MM_MODE == "fp16":
        hidp = hidp.astype(np.float16)
        hidt = hidt.astype(np.float16)
        WF = WF.astype(np.float16)

    hidp = hidp.reshape(B, P, NFULL * H)
    wt_dev = WF.reshape(B, P, NCHUNK * ML)

    nc = _get_nc(MM_MODE)
    in_maps = [
        {
            "hidp": hidp[c * R : (c + 1) * R],
            "hidt": hidt[c * R : (c + 1) * R],
            "wt": wt_dev[c * R : (c + 1) * R],
        }
        for c in range(NCORES)
    ]
    res = run_bass_kernel_spmd(nc, in_maps, list(range(NCORES)), trace=_trace)
    out = np.concatenate([res.results[c]["out"] for c in range(NCORES)], axis=0)
    out = np.ascontiguousarray(out.astype(np.float32))
    if _trace:
        return out, res
    return out
